# Initial kernel scaffold
#
"""AttnBlock (GroupNorm + single-head self-attention + residual) on 8 TRN2 cores.

Data-parallel over batch: each of the 8 NeuronCores runs the full attention
block for 4 of the 32 images. All heavy matmuls run in float32r (TF32-like,
1 cycle/row on the PE) with fp32 PSUM accumulation.

Per-image dataflow (C=512 channels, S=H*W=1024, P=128 partitions):
  x (C,S) -> groupnorm stats (free-dim reduce + indicator matmuls) -> hn (C,S)
  q = wq@hn, k = wk@hn                      (C,S)   [weights host-pretransposed]
  vT = hn^T@wv^T                            (S,C)   [produced transposed by PE]
  sT = k^T-chunks @ q = scores^T            (S2,S1)
  a' = exp(sT * c^-0.5)                     (S2,S1) [unnormalized probs]
  r  = ones^T @ a'  (softmax denominator),  Rb = 1/r broadcast
  o2 = vT-chunks @ a' * Rb                  (C,S1)
  y  = wp@o2 + bp + x                       (C,S)
No transposes and no collectives anywhere.
"""

import numpy as np

import concourse.bass as bass
import concourse.mybir as mybir
import concourse.tile as tile
from concourse import bass_utils
from concourse.bass import ts

# ---------------------------------------------------------------------------
# This container's walrus build accepts at most ONE sync-wait command per
# instruction; Tile routinely attaches several. Split the excess onto
# preceding same-engine NoOps (and extra SP drains for the kernel tail).
# ---------------------------------------------------------------------------
from bass_rust import ScopedClock

_MAX_WAITS = 1


def _drain_and_barrier_split(self, tick_clock, wait_clock):
    drain_inst = self.nc.sync.drain()
    wait_clock.add_sem_waits(
        drain_inst.ins, ScopedClock({None: tick_clock.global_clock})
    )
    si = drain_inst.ins.sync_info
    waits = list(si.on_wait) if si is not None and si.on_wait else []
    if len(waits) > _MAX_WAITS:
        si.on_wait = waits[:_MAX_WAITS]
        drain_inst.ins.sync_info = si
        for i in range(_MAX_WAITS, len(waits), _MAX_WAITS):
            extra = self.nc.sync.drain()
            extra.ins.sync_info = mybir.SyncInfo(
                on_wait=waits[i : i + _MAX_WAITS], on_update=[]
            )
    self.nc.all_engine_barrier()
    assert self.sems is not None
    popped = self.nc._tile_sem_poison_stack.pop()
    assert popped is self._sem_poison
    self.nc.clear_and_free_semaphores(list(self.sems.allocated().values()))
    self.nc.all_engine_barrier()


_orig_add_instruction = tile.TileContext._add_instruction


def _add_instruction_split(self, inst):
    si = inst.sync_info
    if si is not None and si.on_wait and len(si.on_wait) > _MAX_WAITS:
        waits = list(si.on_wait)
        for i in range(0, len(waits) - _MAX_WAITS, _MAX_WAITS):
            nop = mybir.InstNoOp(
                name=f"I-{self.nc.next_id()}", engine=inst.engine, ins=[], outs=[]
            )
            nop.sync_info = mybir.SyncInfo(
                on_wait=waits[i : i + _MAX_WAITS], on_update=[]
            )
            _orig_add_instruction(self, nop)
        si.on_wait = waits[len(waits) - _MAX_WAITS :]
        inst.sync_info = si
    _orig_add_instruction(self, inst)


tile.TileContext._drain_and_barrier = _drain_and_barrier_split
tile.TileContext._add_instruction = _add_instruction_split

# ---------------------------------------------------------------------------

N_CORES = 8
B, C, H, W = 32, 512, 32, 32
S = H * W            # 1024 spatial positions
B_LOC = B // N_CORES  # 4 images per core
P = 128
CI = C // P          # 4 channel chunks
ST = S // P          # 8 spatial tiles (partition side)
NB = 512             # matmul moving free dim / psum bank width
SC = S // NB         # 2 spatial chunks (free side)
GROUPS = 32
GSIZE = C // GROUPS  # 16 channels per group
EPS = 1e-5

F32 = mybir.dt.float32
F32R = mybir.dt.float32r
BF16 = mybir.dt.bfloat16
CDT = BF16  # matmul operand dtype (flip to F32R for higher precision)

TRACE = False
LAST_EXEC_NS = None

_cache = {}


def _build():
    nc = bass.Bass()
    x_ext = nc.declare_dram_parameter("x", [B_LOC, C, S], F32, isOutput=False)
    wT_ext = {
        n: nc.declare_dram_parameter(n, [C, C], F32, isOutput=False)
        for n in ("wqT", "wkT", "wvT", "wpT")
    }
    vec_ext = {
        n: nc.declare_dram_parameter(n, [C], F32, isOutput=False)
        for n in ("bq", "bk", "bv", "bp", "gn_scale", "gn_bias")
    }
    g_ext = nc.declare_dram_parameter("gind", [C, GROUPS], F32, isOutput=False)
    gt_ext = nc.declare_dram_parameter("gindT", [GROUPS, C], F32, isOutput=False)
    out_ext = nc.declare_dram_parameter("out", [B_LOC, C, S], F32, isOutput=True)

    att_scale = float(C) ** -0.5
    inv_gn = 1.0 / float(GSIZE * S)

    with tile.TileContext(nc) as tc, nc.allow_low_precision(
        reason="f32r/bf16 matmul operands; fp32 PSUM accumulation throughout"
    ):
        import contextlib

        ctx = contextlib.ExitStack()
        with ctx:
            consts = ctx.enter_context(tc.tile_pool(name="consts", bufs=1))
            wstage = ctx.enter_context(tc.tile_pool(name="wstage", bufs=1))
            xpool = ctx.enter_context(tc.tile_pool(name="xpool", bufs=2))
            hnpool = ctx.enter_context(tc.tile_pool(name="hnpool", bufs=2))
            qpool = ctx.enter_context(tc.tile_pool(name="qpool", bufs=1))
            o2pool = ctx.enter_context(tc.tile_pool(name="o2pool", bufs=1))
            kpool = ctx.enter_context(tc.tile_pool(name="kpool", bufs=1))
            vtpool = ctx.enter_context(tc.tile_pool(name="vtpool", bufs=1))
            appool = ctx.enter_context(tc.tile_pool(name="appool", bufs=1))
            sqpool = ctx.enter_context(tc.tile_pool(name="sqpool", bufs=1))
            stats = ctx.enter_context(tc.tile_pool(name="stats", bufs=2))
            rbpool = ctx.enter_context(tc.tile_pool(name="rbpool", bufs=1))
            psmm = ctx.enter_context(tc.tile_pool(name="psmm", bufs=5, space="PSUM"))
            psr = ctx.enter_context(tc.tile_pool(name="psr", bufs=1, space="PSUM"))
            psg = ctx.enter_context(tc.tile_pool(name="psg", bufs=1, space="PSUM"))

            bqt = consts.tile([P, CI], F32, tag="bqt")
            nc.gpsimd.dma_start(out=bqt[:], in_=vec_ext["bq"].rearrange("(c p) -> p c", p=P))
            bkt = consts.tile([P, CI], F32, tag="bkt")
            nc.gpsimd.dma_start(out=bkt[:], in_=vec_ext["bk"].rearrange("(c p) -> p c", p=P))
            gsc = consts.tile([P, CI], F32, tag="gsc")
            nc.gpsimd.dma_start(
                out=gsc[:], in_=vec_ext["gn_scale"].rearrange("(c p) -> p c", p=P)
            )
            gbs = consts.tile([P, CI], F32, tag="gbs")
            nc.gpsimd.dma_start(
                out=gbs[:], in_=vec_ext["gn_bias"].rearrange("(c p) -> p c", p=P)
            )

            bpt = consts.tile([P, CI], F32, tag="bpt")
            nc.gpsimd.dma_start(out=bpt[:], in_=vec_ext["bp"].rearrange("(c p) -> p c", p=P))
            bvb = consts.tile([P, C], F32, tag="bvb")
            nc.gpsimd.dma_start(
                out=bvb[:],
                in_=bass.AP(tensor=vec_ext["bv"], offset=0, ap=[[0, P], [1, C]]),
            )


            gm = consts.tile([P, CI, GROUPS], F32, tag="gm")
            nc.gpsimd.dma_start(out=gm[:], in_=g_ext.rearrange("(c p) g -> p c g", p=P))
            gtm = consts.tile([GROUPS, CI, P], F32, tag="gtm")
            nc.gpsimd.dma_start(out=gtm[:], in_=gt_ext.rearrange("g (c p) -> g c p", p=P))

            onestage = wstage.tile([P, NB], F32, tag="onestage")
            nc.vector.memset(onestage[:], 1.0)
            ones_col = consts.tile([P, 1], CDT, tag="ones_col")
            nc.vector.tensor_copy(out=ones_col[:], in_=onestage[:, 0:1])
            ones_row = consts.tile([1, NB], CDT, tag="ones_row")
            nc.vector.tensor_copy(out=ones_row[:], in_=onestage[0:1, :])

            eps32 = consts.tile([GROUPS, 1], F32, tag="eps32")
            nc.vector.memset(eps32[:], EPS)


            # ---- x loads first (image 0's GroupNorm is the startup critical path) ----
            xts = []
            for img in range(B_LOC):
                xt = xpool.tile([P, CI, S], F32, tag="x", name=f"x{img}")
                xsrc = x_ext[img].rearrange("(c p) s -> p c s", p=P)
                for ci in range(CI):
                    nc.sync.dma_start(out=xt[:, ci, :], in_=xsrc[:, ci, :])
                xts.append(xt)

            # ---- weights: loaded lazily, after image 0's GroupNorm stats are
            # emitted, so the Vector-stream ticks image 0 waits on don't
            # include the 16 weight casts ----
            wr = {}
            wsrc = {n: wT_ext[n].rearrange("(c p) o -> p c o", p=P) for n in wT_ext}

            def load_weights():
                for n in ("wqT", "wkT", "wvT", "wpT"):
                    wr[n] = consts.tile([P, CI, C], CDT, tag=f"wr_{n}", name=f"wr_{n}")
                    for ci in range(CI):
                        stg = wstage.tile([P, C], F32, tag="wstg", bufs=8, name="stg")
                        nc.sync.dma_start(out=stg[:], in_=wsrc[n][:, ci, :])
                        nc.vector.tensor_copy(out=wr[n][:, ci, :], in_=stg[:])

            # ---- per image, software-pipelined EMISSION: GroupNorm of
            # image i+1 is emitted before attention of image i, so the Tile
            # scheduler (priority follows emission order) runs it during image
            # i's matmuls instead of serializing it at the image boundary ----

            def emit_gn(img):
                xt = xts[img]
                # GroupNorm stats: per-channel sum + sum of squares
                ssum = stats.tile([P, CI, 2], F32, tag="ssum", name=f"ssum{img}")
                for ci in range(CI):
                    nc.vector.reduce_sum(
                        out=ssum[:, ci, 0:1], in_=xt[:, ci, :], axis=mybir.AxisListType.X
                    )
                    sq = sqpool.tile([P, S], F32, tag="sq", name=f"sq{img}{ci}")
                    nc.scalar.activation(
                        out=sq[:],
                        in_=xt[:, ci, :],
                        func=mybir.ActivationFunctionType.Square,
                        accum_out=ssum[:, ci, 1:2],
                    )
                # group-reduce via indicator matmul: (32, 2) = sum over channels
                pg = psg.tile([GROUPS, 2], F32, tag="gn", name=f"pg{img}")
                for ci in range(CI):
                    nc.tensor.matmul(
                        pg[:],
                        gm[:, ci, :],
                        ssum[:, ci, :],
                        start=(ci == 0),
                        stop=(ci == CI - 1),
                    )
                mv = stats.tile([GROUPS, 2], F32, tag="mv", name=f"mv{img}")
                nc.vector.tensor_scalar_mul(out=mv[:], in0=pg[:], scalar1=inv_gn)
                m2 = stats.tile([GROUPS, 1], F32, tag="m2", name=f"m2{img}")
                nc.vector.tensor_mul(out=m2[:], in0=mv[:, 0:1], in1=mv[:, 0:1])
                var = stats.tile([GROUPS, 1], F32, tag="var", name=f"var{img}")
                nc.vector.tensor_sub(out=var[:], in0=mv[:, 1:2], in1=m2[:])
                std = stats.tile([GROUPS, 1], F32, tag="std", name=f"std{img}")
                nc.scalar.activation(
                    out=std[:],
                    in_=var[:],
                    func=mybir.ActivationFunctionType.Sqrt,
                    bias=eps32[:],
                )
                grp = stats.tile([GROUPS, 2], F32, tag="grp", name=f"grp{img}")
                nc.vector.tensor_scalar_mul(out=grp[:, 0:1], in0=mv[:, 0:1], scalar1=-1.0)
                nc.vector.reciprocal(out=grp[:, 1:2], in_=std[:])

                # expand group stats back to per-channel a, b
                a_t = stats.tile([P, CI], F32, tag="a_t", name=f"a_t{img}")
                b_t = stats.tile([P, CI], F32, tag="b_t", name=f"b_t{img}")
                for ci in range(CI):
                    pe_ = psg.tile([P, 2], F32, tag="gn", name=f"pe{img}{ci}")
                    nc.tensor.matmul(pe_[:], gtm[:, ci, :], grp[:], start=True, stop=True)
                    nc.vector.tensor_mul(
                        out=a_t[:, ci : ci + 1], in0=pe_[:, 1:2], in1=gsc[:, ci : ci + 1]
                    )
                    # b = gn_bias + (-mean)*a   (pe_[:,0:1] holds -mean)
                    nc.vector.scalar_tensor_tensor(
                        out=b_t[:, ci : ci + 1],
                        in0=pe_[:, 0:1],
                        scalar=a_t[:, ci : ci + 1],
                        in1=gbs[:, ci : ci + 1],
                        op0=mybir.AluOpType.mult,
                        op1=mybir.AluOpType.add,
                    )

                # hn = a*x + b; chunks split across Scalar and Vector engines so
                # the four applies don't serialize on one engine
                hn = hnpool.tile([P, CI, S], CDT, tag="hn", name=f"hn{img}")
                for ci in range(CI):
                    if ci % 2 == 0:
                        nc.scalar.activation(
                            out=hn[:, ci, :],
                            in_=xt[:, ci, :],
                            func=mybir.ActivationFunctionType.Identity,
                            bias=b_t[:, ci : ci + 1],
                            scale=a_t[:, ci : ci + 1],
                        )
                    else:
                        nc.vector.tensor_scalar(
                            out=hn[:, ci, :],
                            in0=xt[:, ci, :],
                            scalar1=a_t[:, ci : ci + 1],
                            scalar2=b_t[:, ci : ci + 1],
                            op0=mybir.AluOpType.mult,
                            op1=mybir.AluOpType.add,
                        )
                return hn

            def emit_attn(img, hn):
                xt = xts[img]
                # q, k projections: (C, S)
                qt = qpool.tile([P, CI, S], CDT, tag="q", name=f"q{img}")
                kt = kpool.tile([P, CI, S], CDT, tag="k", name=f"k{img}")
                for w_, bias_t, outt in ((wr["wqT"], bqt, qt), (wr["wkT"], bkt, kt)):
                    for ot in range(CI):
                        for sc in range(SC):
                            pq = psmm.tile([P, NB], F32, tag="mm", name="pq")
                            for ci in range(CI):
                                nc.tensor.matmul(
                                    pq[:],
                                    w_[:, ci, ts(ot, P)],
                                    hn[:, ci, ts(sc, NB)],
                                    start=(ci == 0),
                                    stop=(ci == CI - 1),
                                )
                            nc.vector.tensor_scalar_add(
                                out=outt[:, ot, ts(sc, NB)],
                                in0=pq[:],
                                scalar1=bias_t[:, ot : ot + 1],
                            )

                # v^T: (S, C)
                vt = vtpool.tile([P, ST, C], CDT, tag="vt", name=f"vt{img}")
                for st in range(ST):
                    pv = psmm.tile([P, NB], F32, tag="mm", name="pv")
                    for ci in range(CI):
                        nc.tensor.matmul(
                            pv[:],
                            hn[:, ci, ts(st, P)],
                            wr["wvT"][:, ci, :],
                            start=(ci == 0),
                            stop=(ci == CI - 1),
                        )
                    nc.vector.tensor_add(out=vt[:, st, :], in0=pv[:], in1=bvb[:])

                # scores^T + exp -> unnormalized probs a' (S2, S1)
                ap_ = appool.tile([P, ST, S], CDT, tag="ap", name=f"ap{img}")
                for st in range(ST):
                    for sc in range(SC):
                        psc = psmm.tile([P, NB], F32, tag="mm", name="psc")
                        for ci in range(CI):
                            nc.tensor.matmul(
                                psc[:],
                                kt[:, ci, ts(st, P)],
                                qt[:, ci, ts(sc, NB)],
                                start=(ci == 0),
                                stop=(ci == CI - 1),
                            )
                        nc.scalar.activation(
                            out=ap_[:, st, ts(sc, NB)],
                            in_=psc[:],
                            func=mybir.ActivationFunctionType.Exp,
                            scale=att_scale,
                        )

                # softmax denominators r (PE) -> 1/r (DVE, overlaps AV matmuls)
                rrec = rbpool.tile([1, S], F32, tag="rrec", name=f"rrec{img}")
                rrec_c = rbpool.tile([1, S], CDT, tag="rrec_c", name=f"rrc{img}")
                rb = rbpool.tile([P, S], F32, tag="rb", name=f"rb{img}")
                for sc in range(SC):
                    pr = psr.tile([1, NB], F32, tag="r", name=f"pr{img}{sc}", bufs=2)
                    for st in range(ST):
                        nc.tensor.matmul(
                            pr[:],
                            ones_col[:],
                            ap_[:, st, ts(sc, NB)],
                            start=(st == 0),
                            stop=(st == ST - 1),
                        )
                    nc.vector.reciprocal(out=rrec[0:1, ts(sc, NB)], in_=pr[:])
                    nc.vector.tensor_copy(
                        out=rrec_c[0:1, ts(sc, NB)], in_=rrec[0:1, ts(sc, NB)]
                    )

                # attention output (unnormalized): po = vt-chunks @ a'.
                # The 1/r broadcast matmul for chunk sc is emitted right after
                # sc's AV groups so the normalize-evacuations overlap the other
                # chunk's matmuls and projection never waits.
                o2 = o2pool.tile([P, CI, S], CDT, tag="o2", name=f"o2{img}")
                for sc in range(SC):
                    pos = []
                    for ct in range(CI):
                        po = psmm.tile([P, NB], F32, tag="mm", name=f"po{ct}{sc}")
                        for st in range(ST):
                            nc.tensor.matmul(
                                po[:],
                                vt[:, st, ts(ct, P)],
                                ap_[:, st, ts(sc, NB)],
                                start=(st == 0),
                                stop=(st == ST - 1),
                            )
                        pos.append((ct, po))
                    rbps = psr.tile([P, NB], F32, tag="r", name=f"rbps{img}{sc}", bufs=2)
                    nc.tensor.matmul(
                        rbps[:],
                        ones_row[0:1, 0:P],
                        rrec_c[0:1, ts(sc, NB)],
                        start=True,
                        stop=True,
                    )
                    nc.vector.tensor_copy(out=rb[:, ts(sc, NB)], in_=rbps[:])
                    for ct, po in pos:
                        nc.vector.tensor_mul(
                            out=o2[:, ct, ts(sc, NB)], in0=po[:], in1=rb[:, ts(sc, NB)]
                        )

                # projection + bias + residual (single fused DVE op per chunk)
                for sc in range(SC):
                    for ot in range(CI):
                        pp = psmm.tile([P, NB], F32, tag="mm", name="pp")
                        for ci in range(CI):
                            nc.tensor.matmul(
                                pp[:],
                                wr["wpT"][:, ci, ts(ot, P)],
                                o2[:, ci, ts(sc, NB)],
                                start=(ci == 0),
                                stop=(ci == CI - 1),
                            )
                        nc.vector.scalar_tensor_tensor(
                            out=xt[:, ot, ts(sc, NB)],
                            in0=pp[:],
                            scalar=bpt[:, ot : ot + 1],
                            in1=xt[:, ot, ts(sc, NB)],
                            op0=mybir.AluOpType.add,
                            op1=mybir.AluOpType.add,
                        )
                for ot in range(CI):
                    if img == B_LOC - 1:
                        for sc in range(SC):
                            nc.sync.dma_start(
                                out=out_ext[img, ot * P : (ot + 1) * P, ts(sc, NB)],
                                in_=xt[:, ot, ts(sc, NB)],
                            )
                    else:
                        nc.sync.dma_start(
                            out=out_ext[img, ot * P : (ot + 1) * P, :],
                            in_=xt[:, ot, :],
                        )

            hns = {0: emit_gn(0)}
            load_weights()
            for img in range(B_LOC):
                if img + 1 < B_LOC:
                    hns[img + 1] = emit_gn(img + 1)
                emit_attn(img, hns.pop(img))
    return nc


def _prep_inputs(x, gn_scale, gn_bias, wq, bq, wk, bk, wv, bv, wp, bp):
    f = lambda a: np.ascontiguousarray(np.asarray(a, dtype=np.float32))
    x = f(x).reshape(B, C, S)
    shared = {
        "wqT": f(np.asarray(wq).T),
        "wkT": f(np.asarray(wk).T),
        "wvT": f(np.asarray(wv).T),
        "wpT": f(np.asarray(wp).T),
        "bq": f(bq),
        "bk": f(bk),
        "bv": f(bv),
        "bp": f(bp),
        "gn_scale": f(gn_scale),
        "gn_bias": f(gn_bias),
        "gind": np.eye(GROUPS, dtype=np.float32).repeat(GSIZE, axis=0),
        "gindT": np.ascontiguousarray(
            np.eye(GROUPS, dtype=np.float32).repeat(GSIZE, axis=0).T
        ),
    }
    in_maps = []
    for core in range(N_CORES):
        m = dict(shared)
        m["x"] = np.ascontiguousarray(x[core * B_LOC : (core + 1) * B_LOC])
        in_maps.append(m)
    return in_maps


def kernel(x, gn_scale, gn_bias, wq, bq, wk, bk, wv, bv, wp, bp):
    global LAST_EXEC_NS
    if "nc" not in _cache:
        _cache["nc"] = _build()
    nc = _cache["nc"]
    in_maps = _prep_inputs(x, gn_scale, gn_bias, wq, bq, wk, bk, wv, bv, wp, bp)
    res = bass_utils.run_bass_kernel_spmd(
        nc, in_maps, core_ids=list(range(N_CORES)), trace=TRACE
    )
    LAST_EXEC_NS = res.exec_time_ns
    out = np.concatenate([res.results[i]["out"] for i in range(N_CORES)], axis=0)
    return out.reshape(B, C, H, W)



# revision 1
# speedup vs baseline: 1.0079x; 1.0079x over previous
"""AttnBlock (GroupNorm + single-head self-attention + residual) on 8 TRN2 cores.

Data-parallel over batch: each of the 8 NeuronCores runs the full attention
block for 4 of the 32 images. All heavy matmuls run in float32r (TF32-like,
1 cycle/row on the PE) with fp32 PSUM accumulation.

Per-image dataflow (C=512 channels, S=H*W=1024, P=128 partitions):
  x (C,S) -> groupnorm stats (free-dim reduce + indicator matmuls) -> hn (C,S)
  q = wq@hn, k = wk@hn                      (C,S)   [weights host-pretransposed]
  vT = hn^T@wv^T                            (S,C)   [produced transposed by PE]
  sT = k^T-chunks @ q = scores^T            (S2,S1)
  a' = exp(sT * c^-0.5)                     (S2,S1) [unnormalized probs]
  r  = ones^T @ a'  (softmax denominator),  Rb = 1/r broadcast
  o2 = vT-chunks @ a' * Rb                  (C,S1)
  y  = wp@o2 + bp + x                       (C,S)
No transposes and no collectives anywhere.
"""

import numpy as np

import concourse.bass as bass
import concourse.mybir as mybir
import concourse.tile as tile
from concourse import bass_utils
from concourse.bass import ts

# ---------------------------------------------------------------------------
# This container's walrus build accepts at most ONE sync-wait command per
# instruction; Tile routinely attaches several. Split the excess onto
# preceding same-engine NoOps (and extra SP drains for the kernel tail).
# ---------------------------------------------------------------------------
from bass_rust import ScopedClock

_MAX_WAITS = 1


def _drain_and_barrier_split(self, tick_clock, wait_clock):
    drain_inst = self.nc.sync.drain()
    wait_clock.add_sem_waits(
        drain_inst.ins, ScopedClock({None: tick_clock.global_clock})
    )
    si = drain_inst.ins.sync_info
    waits = list(si.on_wait) if si is not None and si.on_wait else []
    if len(waits) > _MAX_WAITS:
        si.on_wait = waits[:_MAX_WAITS]
        drain_inst.ins.sync_info = si
        for i in range(_MAX_WAITS, len(waits), _MAX_WAITS):
            extra = self.nc.sync.drain()
            extra.ins.sync_info = mybir.SyncInfo(
                on_wait=waits[i : i + _MAX_WAITS], on_update=[]
            )
    self.nc.all_engine_barrier()
    assert self.sems is not None
    popped = self.nc._tile_sem_poison_stack.pop()
    assert popped is self._sem_poison
    self.nc.clear_and_free_semaphores(list(self.sems.allocated().values()))
    self.nc.all_engine_barrier()


_orig_add_instruction = tile.TileContext._add_instruction


def _add_instruction_split(self, inst):
    si = inst.sync_info
    if si is not None and si.on_wait and len(si.on_wait) > _MAX_WAITS:
        waits = list(si.on_wait)
        for i in range(0, len(waits) - _MAX_WAITS, _MAX_WAITS):
            nop = mybir.InstNoOp(
                name=f"I-{self.nc.next_id()}", engine=inst.engine, ins=[], outs=[]
            )
            nop.sync_info = mybir.SyncInfo(
                on_wait=waits[i : i + _MAX_WAITS], on_update=[]
            )
            _orig_add_instruction(self, nop)
        si.on_wait = waits[len(waits) - _MAX_WAITS :]
        inst.sync_info = si
    _orig_add_instruction(self, inst)


tile.TileContext._drain_and_barrier = _drain_and_barrier_split
tile.TileContext._add_instruction = _add_instruction_split

# ---------------------------------------------------------------------------

N_CORES = 8
B, C, H, W = 32, 512, 32, 32
S = H * W            # 1024 spatial positions
B_LOC = B // N_CORES  # 4 images per core
P = 128
CI = C // P          # 4 channel chunks
ST = S // P          # 8 spatial tiles (partition side)
NB = 512             # matmul moving free dim / psum bank width
SC = S // NB         # 2 spatial chunks (free side)
GROUPS = 32
GSIZE = C // GROUPS  # 16 channels per group
EPS = 1e-5

F32 = mybir.dt.float32
F32R = mybir.dt.float32r
BF16 = mybir.dt.bfloat16
CDT = BF16  # matmul operand dtype (flip to F32R for higher precision)

TRACE = False
LAST_EXEC_NS = None

_cache = {}


def _build():
    nc = bass.Bass()
    x_ext = nc.declare_dram_parameter("x", [B_LOC, C, S], F32, isOutput=False)
    wT_ext = {
        n: nc.declare_dram_parameter(n, [C, C], F32, isOutput=False)
        for n in ("wqT", "wkT", "wvT", "wpT")
    }
    vec_ext = {
        n: nc.declare_dram_parameter(n, [C], F32, isOutput=False)
        for n in ("bq", "bk", "bv", "bp", "gn_scale", "gn_bias")
    }
    g_ext = nc.declare_dram_parameter("gind", [C, GROUPS], F32, isOutput=False)
    gt_ext = nc.declare_dram_parameter("gindT", [GROUPS, C], F32, isOutput=False)
    out_ext = nc.declare_dram_parameter("out", [B_LOC, C, S], F32, isOutput=True)

    att_scale = float(C) ** -0.5
    inv_gn = 1.0 / float(GSIZE * S)

    with tile.TileContext(nc) as tc, nc.allow_low_precision(
        reason="f32r/bf16 matmul operands; fp32 PSUM accumulation throughout"
    ):
        import contextlib

        ctx = contextlib.ExitStack()
        with ctx:
            consts = ctx.enter_context(tc.tile_pool(name="consts", bufs=1))
            wstage = ctx.enter_context(tc.tile_pool(name="wstage", bufs=1))
            xpool = ctx.enter_context(tc.tile_pool(name="xpool", bufs=2))
            hnpool = ctx.enter_context(tc.tile_pool(name="hnpool", bufs=2))
            qpool = ctx.enter_context(tc.tile_pool(name="qpool", bufs=1))
            o2pool = ctx.enter_context(tc.tile_pool(name="o2pool", bufs=1))
            kpool = ctx.enter_context(tc.tile_pool(name="kpool", bufs=1))
            vtpool = ctx.enter_context(tc.tile_pool(name="vtpool", bufs=1))
            appool = ctx.enter_context(tc.tile_pool(name="appool", bufs=1))
            sqpool = ctx.enter_context(tc.tile_pool(name="sqpool", bufs=1))
            stats = ctx.enter_context(tc.tile_pool(name="stats", bufs=2))
            rbpool = ctx.enter_context(tc.tile_pool(name="rbpool", bufs=1))
            psmm = ctx.enter_context(tc.tile_pool(name="psmm", bufs=5, space="PSUM"))
            psr = ctx.enter_context(tc.tile_pool(name="psr", bufs=1, space="PSUM"))
            psg = ctx.enter_context(tc.tile_pool(name="psg", bufs=1, space="PSUM"))

            bqt = consts.tile([P, CI], F32, tag="bqt")
            nc.gpsimd.dma_start(out=bqt[:], in_=vec_ext["bq"].rearrange("(c p) -> p c", p=P))
            bkt = consts.tile([P, CI], F32, tag="bkt")
            nc.gpsimd.dma_start(out=bkt[:], in_=vec_ext["bk"].rearrange("(c p) -> p c", p=P))
            gsc = consts.tile([P, CI], F32, tag="gsc")
            nc.gpsimd.dma_start(
                out=gsc[:], in_=vec_ext["gn_scale"].rearrange("(c p) -> p c", p=P)
            )
            gbs = consts.tile([P, CI], F32, tag="gbs")
            nc.gpsimd.dma_start(
                out=gbs[:], in_=vec_ext["gn_bias"].rearrange("(c p) -> p c", p=P)
            )

            bpt = consts.tile([P, CI], F32, tag="bpt")
            nc.gpsimd.dma_start(out=bpt[:], in_=vec_ext["bp"].rearrange("(c p) -> p c", p=P))
            bvb = consts.tile([P, C], F32, tag="bvb")
            nc.gpsimd.dma_start(
                out=bvb[:],
                in_=bass.AP(tensor=vec_ext["bv"], offset=0, ap=[[0, P], [1, C]]),
            )


            gm = consts.tile([P, CI, GROUPS], F32, tag="gm")
            nc.gpsimd.dma_start(out=gm[:], in_=g_ext.rearrange("(c p) g -> p c g", p=P))
            gtm = consts.tile([GROUPS, CI, P], F32, tag="gtm")
            nc.gpsimd.dma_start(out=gtm[:], in_=gt_ext.rearrange("g (c p) -> g c p", p=P))

            onestage = wstage.tile([P, NB], F32, tag="onestage")
            nc.vector.memset(onestage[:], 1.0)
            ones_col = consts.tile([P, 1], CDT, tag="ones_col")
            nc.vector.tensor_copy(out=ones_col[:], in_=onestage[:, 0:1])
            ones_row = consts.tile([1, NB], CDT, tag="ones_row")
            nc.vector.tensor_copy(out=ones_row[:], in_=onestage[0:1, :])

            eps32 = consts.tile([GROUPS, 1], F32, tag="eps32")
            nc.vector.memset(eps32[:], EPS)


            # ---- x loads first (image 0's GroupNorm is the startup critical path) ----
            xts = []
            for img in range(B_LOC):
                xt = xpool.tile([P, CI, S], F32, tag="x", name=f"x{img}")
                xsrc = x_ext[img].rearrange("(c p) s -> p c s", p=P)
                for ci in range(CI):
                    nc.sync.dma_start(out=xt[:, ci, :], in_=xsrc[:, ci, :])
                xts.append(xt)

            # ---- weights: loaded lazily, after image 0's GroupNorm stats are
            # emitted, so the Vector-stream ticks image 0 waits on don't
            # include the 16 weight casts ----
            wr = {}
            wsrc = {n: wT_ext[n].rearrange("(c p) o -> p c o", p=P) for n in wT_ext}

            def load_weights():
                for n in ("wqT", "wkT", "wvT", "wpT"):
                    wr[n] = consts.tile([P, CI, C], CDT, tag=f"wr_{n}", name=f"wr_{n}")
                    for ci in range(CI):
                        stg = wstage.tile([P, C], F32, tag="wstg", bufs=8, name="stg")
                        nc.sync.dma_start(out=stg[:], in_=wsrc[n][:, ci, :])
                        nc.vector.tensor_copy(out=wr[n][:, ci, :], in_=stg[:])

            # ---- per image, software-pipelined EMISSION: GroupNorm of
            # image i+1 is emitted before attention of image i, so the Tile
            # scheduler (priority follows emission order) runs it during image
            # i's matmuls instead of serializing it at the image boundary ----

            def emit_gn(img):
                xt = xts[img]
                # GroupNorm stats: per-channel sum + sum of squares
                ssum = stats.tile([P, CI, 2], F32, tag="ssum", name=f"ssum{img}")
                for ci in range(CI):
                    nc.vector.reduce_sum(
                        out=ssum[:, ci, 0:1], in_=xt[:, ci, :], axis=mybir.AxisListType.X
                    )
                    sq = sqpool.tile([P, S], F32, tag="sq", name=f"sq{img}{ci}")
                    nc.scalar.activation(
                        out=sq[:],
                        in_=xt[:, ci, :],
                        func=mybir.ActivationFunctionType.Square,
                        accum_out=ssum[:, ci, 1:2],
                    )
                # group-reduce via indicator matmul: (32, 2) = sum over channels
                pg = psg.tile([GROUPS, 2], F32, tag="gn", name=f"pg{img}")
                for ci in range(CI):
                    nc.tensor.matmul(
                        pg[:],
                        gm[:, ci, :],
                        ssum[:, ci, :],
                        start=(ci == 0),
                        stop=(ci == CI - 1),
                    )
                mv = stats.tile([GROUPS, 2], F32, tag="mv", name=f"mv{img}")
                nc.vector.tensor_scalar_mul(out=mv[:], in0=pg[:], scalar1=inv_gn)
                m2 = stats.tile([GROUPS, 1], F32, tag="m2", name=f"m2{img}")
                nc.vector.tensor_mul(out=m2[:], in0=mv[:, 0:1], in1=mv[:, 0:1])
                var = stats.tile([GROUPS, 1], F32, tag="var", name=f"var{img}")
                nc.vector.tensor_sub(out=var[:], in0=mv[:, 1:2], in1=m2[:])
                std = stats.tile([GROUPS, 1], F32, tag="std", name=f"std{img}")
                nc.scalar.activation(
                    out=std[:],
                    in_=var[:],
                    func=mybir.ActivationFunctionType.Sqrt,
                    bias=eps32[:],
                )
                grp = stats.tile([GROUPS, 2], F32, tag="grp", name=f"grp{img}")
                nc.vector.tensor_scalar_mul(out=grp[:, 0:1], in0=mv[:, 0:1], scalar1=-1.0)
                nc.vector.reciprocal(out=grp[:, 1:2], in_=std[:])

                # expand group stats back to per-channel a, b
                a_t = stats.tile([P, CI], F32, tag="a_t", name=f"a_t{img}")
                b_t = stats.tile([P, CI], F32, tag="b_t", name=f"b_t{img}")
                for ci in range(CI):
                    pe_ = psg.tile([P, 2], F32, tag="gn", name=f"pe{img}{ci}")
                    nc.tensor.matmul(pe_[:], gtm[:, ci, :], grp[:], start=True, stop=True)
                    nc.vector.tensor_mul(
                        out=a_t[:, ci : ci + 1], in0=pe_[:, 1:2], in1=gsc[:, ci : ci + 1]
                    )
                    # b = gn_bias + (-mean)*a   (pe_[:,0:1] holds -mean)
                    nc.vector.scalar_tensor_tensor(
                        out=b_t[:, ci : ci + 1],
                        in0=pe_[:, 0:1],
                        scalar=a_t[:, ci : ci + 1],
                        in1=gbs[:, ci : ci + 1],
                        op0=mybir.AluOpType.mult,
                        op1=mybir.AluOpType.add,
                    )

                # hn = a*x + b; chunks split across Scalar and Vector engines so
                # the four applies don't serialize on one engine
                hn = hnpool.tile([P, CI, S], CDT, tag="hn", name=f"hn{img}")
                for ci in range(CI):
                    if ci % 2 == 0:
                        nc.scalar.activation(
                            out=hn[:, ci, :],
                            in_=xt[:, ci, :],
                            func=mybir.ActivationFunctionType.Identity,
                            bias=b_t[:, ci : ci + 1],
                            scale=a_t[:, ci : ci + 1],
                        )
                    else:
                        nc.vector.tensor_scalar(
                            out=hn[:, ci, :],
                            in0=xt[:, ci, :],
                            scalar1=a_t[:, ci : ci + 1],
                            scalar2=b_t[:, ci : ci + 1],
                            op0=mybir.AluOpType.mult,
                            op1=mybir.AluOpType.add,
                        )
                return hn

            def emit_attn(img, hn):
                xt = xts[img]
                # q, k projections: (C, S)
                qt = qpool.tile([P, CI, S], CDT, tag="q", name=f"q{img}")
                kt = kpool.tile([P, CI, S], CDT, tag="k", name=f"k{img}")
                for w_, bias_t, outt in ((wr["wqT"], bqt, qt), (wr["wkT"], bkt, kt)):
                    for ot in range(CI):
                        for sc in range(SC):
                            pq = psmm.tile([P, NB], F32, tag="mm", name="pq")
                            for ci in range(CI):
                                nc.tensor.matmul(
                                    pq[:],
                                    w_[:, ci, ts(ot, P)],
                                    hn[:, ci, ts(sc, NB)],
                                    start=(ci == 0),
                                    stop=(ci == CI - 1),
                                )
                            nc.vector.tensor_scalar_add(
                                out=outt[:, ot, ts(sc, NB)],
                                in0=pq[:],
                                scalar1=bias_t[:, ot : ot + 1],
                            )

                # v^T: (S, C)
                vt = vtpool.tile([P, ST, C], CDT, tag="vt", name=f"vt{img}")
                for st in range(ST):
                    pv = psmm.tile([P, NB], F32, tag="mm", name="pv")
                    for ci in range(CI):
                        nc.tensor.matmul(
                            pv[:],
                            hn[:, ci, ts(st, P)],
                            wr["wvT"][:, ci, :],
                            start=(ci == 0),
                            stop=(ci == CI - 1),
                        )
                    nc.vector.tensor_add(out=vt[:, st, :], in0=pv[:], in1=bvb[:])

                # scores^T + exp -> unnormalized probs a' (S2, S1)
                ap_ = appool.tile([P, ST, S], CDT, tag="ap", name=f"ap{img}")
                for st in range(ST):
                    for sc in range(SC):
                        psc = psmm.tile([P, NB], F32, tag="mm", name="psc")
                        for ci in range(CI):
                            nc.tensor.matmul(
                                psc[:],
                                kt[:, ci, ts(st, P)],
                                qt[:, ci, ts(sc, NB)],
                                start=(ci == 0),
                                stop=(ci == CI - 1),
                            )
                        nc.scalar.activation(
                            out=ap_[:, st, ts(sc, NB)],
                            in_=psc[:],
                            func=mybir.ActivationFunctionType.Exp,
                            scale=att_scale,
                        )

                # softmax denominators r (PE) -> 1/r (DVE, overlaps AV matmuls)
                rrec = rbpool.tile([1, S], F32, tag="rrec", name=f"rrec{img}")
                rrec_c = rbpool.tile([1, S], CDT, tag="rrec_c", name=f"rrc{img}")
                rb = rbpool.tile([P, S], F32, tag="rb", name=f"rb{img}")
                for sc in range(SC):
                    pr = psr.tile([1, NB], F32, tag="r", name=f"pr{img}{sc}", bufs=2)
                    for st in range(ST):
                        nc.tensor.matmul(
                            pr[:],
                            ones_col[:],
                            ap_[:, st, ts(sc, NB)],
                            start=(st == 0),
                            stop=(st == ST - 1),
                        )
                    nc.vector.reciprocal(out=rrec[0:1, ts(sc, NB)], in_=pr[:])
                    nc.vector.tensor_copy(
                        out=rrec_c[0:1, ts(sc, NB)], in_=rrec[0:1, ts(sc, NB)]
                    )

                # attention output (unnormalized): po = vt-chunks @ a'.
                # The 1/r broadcast matmul for chunk sc is emitted right after
                # sc's AV groups so the normalize-evacuations overlap the other
                # chunk's matmuls and projection never waits.
                o2 = o2pool.tile([P, CI, S], CDT, tag="o2", name=f"o2{img}")
                for sc in range(SC):
                    pos = []
                    for ct in range(CI):
                        po = psmm.tile([P, NB], F32, tag="mm", name=f"po{ct}{sc}")
                        for st in range(ST):
                            nc.tensor.matmul(
                                po[:],
                                vt[:, st, ts(ct, P)],
                                ap_[:, st, ts(sc, NB)],
                                start=(st == 0),
                                stop=(st == ST - 1),
                            )
                        pos.append((ct, po))
                    rbps = psr.tile([P, NB], F32, tag="r", name=f"rbps{img}{sc}", bufs=2)
                    nc.tensor.matmul(
                        rbps[:],
                        ones_row[0:1, 0:P],
                        rrec_c[0:1, ts(sc, NB)],
                        start=True,
                        stop=True,
                    )
                    nc.vector.tensor_copy(out=rb[:, ts(sc, NB)], in_=rbps[:])
                    for ct, po in pos:
                        nc.vector.tensor_mul(
                            out=o2[:, ct, ts(sc, NB)], in0=po[:], in1=rb[:, ts(sc, NB)]
                        )

                # projection + bias + residual (single fused DVE op per chunk)
                for sc in range(SC):
                    for ot in range(CI):
                        pp = psmm.tile([P, NB], F32, tag="mm", name="pp")
                        for ci in range(CI):
                            nc.tensor.matmul(
                                pp[:],
                                wr["wpT"][:, ci, ts(ot, P)],
                                o2[:, ci, ts(sc, NB)],
                                start=(ci == 0),
                                stop=(ci == CI - 1),
                            )
                        nc.vector.scalar_tensor_tensor(
                            out=xt[:, ot, ts(sc, NB)],
                            in0=pp[:],
                            scalar=bpt[:, ot : ot + 1],
                            in1=xt[:, ot, ts(sc, NB)],
                            op0=mybir.AluOpType.add,
                            op1=mybir.AluOpType.add,
                        )
                for ot in range(CI):
                    if img == B_LOC - 1:
                        for sc in range(SC):
                            nc.sync.dma_start(
                                out=out_ext[img, ot * P : (ot + 1) * P, ts(sc, NB)],
                                in_=xt[:, ot, ts(sc, NB)],
                            )
                    else:
                        nc.sync.dma_start(
                            out=out_ext[img, ot * P : (ot + 1) * P, :],
                            in_=xt[:, ot, :],
                        )

            hns = {0: emit_gn(0)}
            load_weights()
            for img in range(B_LOC):
                if img + 1 < B_LOC:
                    hns[img + 1] = emit_gn(img + 1)
                emit_attn(img, hns.pop(img))
    return nc


def _prep_inputs(x, gn_scale, gn_bias, wq, bq, wk, bk, wv, bv, wp, bp):
    f = lambda a: np.ascontiguousarray(np.asarray(a, dtype=np.float32))
    x = f(x).reshape(B, C, S)
    shared = {
        "wqT": f(np.asarray(wq).T),
        "wkT": f(np.asarray(wk).T),
        "wvT": f(np.asarray(wv).T),
        "wpT": f(np.asarray(wp).T),
        "bq": f(bq),
        "bk": f(bk),
        "bv": f(bv),
        "bp": f(bp),
        "gn_scale": f(gn_scale),
        "gn_bias": f(gn_bias),
        "gind": np.eye(GROUPS, dtype=np.float32).repeat(GSIZE, axis=0),
        "gindT": np.ascontiguousarray(
            np.eye(GROUPS, dtype=np.float32).repeat(GSIZE, axis=0).T
        ),
    }
    in_maps = []
    for core in range(N_CORES):
        m = dict(shared)
        m["x"] = np.ascontiguousarray(x[core * B_LOC : (core + 1) * B_LOC])
        in_maps.append(m)
    return in_maps


def kernel(x, gn_scale, gn_bias, wq, bq, wk, bk, wv, bv, wp, bp):
    global LAST_EXEC_NS
    if "nc" not in _cache:
        _cache["nc"] = _build()
    nc = _cache["nc"]
    in_maps = _prep_inputs(x, gn_scale, gn_bias, wq, bq, wk, bk, wv, bv, wp, bp)
    res = bass_utils.run_bass_kernel_spmd(
        nc, in_maps, core_ids=list(range(N_CORES)), trace=TRACE
    )
    LAST_EXEC_NS = res.exec_time_ns
    out = np.concatenate([res.results[i]["out"] for i in range(N_CORES)], axis=0)
    return out.reshape(B, C, H, W)

